# revision 21
# baseline (speedup 1.0000x reference)
"""Trainium2 Bass kernel for the Sobel/gabor depthwise-conv + elementwise chain.

reference:
    gx = depthwise3x3(x, KX); gy = depthwise3x3(x, KY)       # SAME zero-pad
    d  = x + 0.001
    gabor = arctan(sqrt((gx/d)^2 + (gy/d)^2)) / 255
    gabor = (gabor - MEAN[c]) / STD[c]
    return (gabor, x)

Kernel v2 strategy (pure data parallel, batch 32 -> 8 cores x 4 images):
  * KX = a_v (x) b_h, KY = c_v (x) a_h with a=[s,1,s], b=[-1,0,1], c=[1,0,-1].
    The vertical convs run on TensorE as tridiagonal band matmuls (bf16,
    2.4 G col/s); the horizontal +-1 shifts are free-dim slices of a
    zero-padded SBUF tile used directly as the moving operands:
       gx = A@x+  + (-A)@x-          (A = tridiag[s,1,s])
       gy = sC@x- + C@x0 + sC@x+     (C = tridiag[1,0,-1])
  * Elementwise tail avoids both ACT Square passes and the Reciprocal table:
       n  = max(|gx|,|gy|)           DVE tensor_tensor abs_max, PSUM->SBUF
       r ~= 1/(1.19*n)               DVE int16 bit-hack: MAGIC - bits(n)
       v  = x * r                    DVE tensor_tensor (2x mode, aligned)
       g  = Arctan(v) -> fp8         ACT (single table set, no switches)
    atan(sqrt(gx^2+gy^2)/d) = pi/2 - atan(d/norm); the L-inf norm (x1.19),
    the ~5% bit-hack reciprocal error and dropping the +0.001 d-offset are
    all far inside the 2e-2 gate (measured ~1.3e-3 in simulation; the atan
    term is bounded so worst-case scale-rel error <= 1.35e-2).
  * pi/2 flip + per-channel (x-MEAN)/STD affine fold into a host-side
    per-channel linear map applied to the fp8 result.
  * H=512 rows: 4 row-tiles of 128 input rows at R0=126j (output rows
    0..126 / 127..252 / 253..378 / 379..504, band variants encode the
    zero padding) plus the 12 groups' last 8 rows packed into one
    [96,512] "tail" tile with block-diagonal 8x8 bottom bands
    (output rows 505..511) so the tail costs ~2%, not 25%.
  * DMA: one big strided dma_start per 2 groups in/out (packets spread
    across all 16 SDMA engines), fp8 output halves write traffic.
"""

import dataclasses
import numpy as np
from contextlib import ExitStack

N_FULL, C, H, W = 32, 3, 512, 512
N_CORES = 8
NPC = N_FULL // N_CORES          # images per core
G = NPC * C                      # 12 groups (n,c) per core
S = 1.0 / (2.0 * np.sqrt(2.0))
MEAN = (0.485, 0.456, 0.406)
STD = (0.229, 0.224, 0.225)

MAGIC = 0x7ED4                   # bf16-bit reciprocal of 1.19*n (numerically opt.)
BLK = 516                        # padded block pitch (2 zero cols each side)
NT = 4                           # full row-tiles per group
TAIL_IN0 = 504                   # tail tile input rows 504..511
ROWS_PG = H                      # 512
ACT_DRAIN = (1,)               # tiles whose gy drain also runs on ScalarE


def make_bands() -> np.ndarray:
    """[128, 1408] bf16 stationary band matrices.

    col blocks of 128: 0:A_top 1:A_int 2:nA_top 3:nA_int 4:C_top 5:C_int
                       6:sC_top 7:sC_int; then 4 tail blocks of 96 at 1024+:
                       A_bot, nA_bot, C_bot, sC_bot (12 block-diag 8x8).
    B[k, m] = weight of input partition k for output partition m.
    top: valid m 0..126 (k=-1 tap dropped); int: valid m 1..126;
    bot blocks: valid m' 1..7 (k'=8 tap dropped).
    """
    import ml_dtypes
    a = np.array([S, 1.0, S], np.float32)     # vertical [s,1,s]
    c = np.array([1.0, 0.0, -1.0], np.float32)  # vertical [1,0,-1]
    out = np.zeros((128, 1408), np.float32)

    def fill(blkcol, wv, mlo, mhi, kmax):
        b = out[:, blkcol:blkcol + 128]
        for m in range(mlo, mhi + 1):
            for d in range(3):
                k = m + d - 1
                if 0 <= k <= kmax:
                    b[k, m] = wv[d]

    for i, wv in enumerate((a, -a, c, S * c)):
        fill((2 * i + 0) * 128, wv, 0, 126, 127)   # top
        fill((2 * i + 1) * 128, wv, 1, 126, 127)   # interior
    # tail bottom: 12 diag blocks of 8x8, valid m' 1..7, taps clipped at 7
    for i, wv in enumerate((a, -a, c, S * c)):
        tb = out[:, 1024 + i * 96:1024 + (i + 1) * 96]
        for g in range(12):
            for m in range(1, 8):
                for d in range(3):
                    k = m + d - 1
                    if 0 <= k <= 7:
                        tb[8 * g + k, 8 * g + m] = wv[d]
    return out.astype(ml_dtypes.bfloat16)


def build_nc():
    from concourse import bacc, mybir, tile

    f32 = mybir.dt.float32
    bf16 = mybir.dt.bfloat16
    i16 = mybir.dt.int16
    fp8 = mybir.dt.float8e4
    AF = mybir.ActivationFunctionType
    ALU = mybir.AluOpType

    nc = bacc.Bacc("TRN2", target_bir_lowering=False, debug=False)
    x_d = nc.declare_dram_parameter("x", [G * H, W], bf16, isOutput=False)
    b_d = nc.declare_dram_parameter("bands", [128, 1408], bf16, isOutput=False)
    o_d = nc.declare_dram_parameter("gabor", [G * H, W], mybir.dt.uint8, isOutput=True)

    def dram_ap(base, dims, offset):
        """Custom strided DRAM AP: dims = [[stride_elems, num], ...]."""
        return dataclasses.replace(base, ap=[list(d) for d in dims],
                                   offset=offset)

    GP = NT * BLK                 # per-group padded pitch (elems) in x tile

    with tile.TileContext(nc) as tc, ExitStack() as ctx:
        cpool = ctx.enter_context(tc.tile_pool(name="const", bufs=1))
        xpool = ctx.enter_context(tc.tile_pool(name="x", bufs=3))
        npool = ctx.enter_context(tc.tile_pool(name="n", bufs=3))
        rpool = ctx.enter_context(tc.tile_pool(name="r", bufs=3))
        vpool = ctx.enter_context(tc.tile_pool(name="v", bufs=3))
        opool = ctx.enter_context(tc.tile_pool(name="o", bufs=3))
        gpool = ctx.enter_context(tc.tile_pool(name="g", bufs=3))
        ppool = ctx.enter_context(tc.tile_pool(name="ps", bufs=2, space="PSUM"))

        bands = cpool.tile([128, 1408], bf16)
        nc.sync.dma_start(out=bands[:], in_=b_d[:, :])

        def band(i, var):
            # i: 0=A 1=nA 2=C 3=sC ; var: 0=top 1=interior
            off = (2 * i + var) * 128
            return bands[0:128, off:off + 128]

        def tband(i):
            return bands[0:96, 1024 + i * 96:1024 + i * 96 + 96]

        mm = nc.tensor.matmul

        def conv_tile(ps, gxo, gyo, xmh, xc, xph, bA, bnA, bC, bsC):
            """5 band matmuls for one row tile into psum slices gxo/gyo."""
            mm(ps[:, gxo[0]:gxo[1]], bA, xph, start=True, stop=False,
               skip_group_check=True)
            mm(ps[:, gxo[0]:gxo[1]], bnA, xmh, start=False, stop=True,
               skip_group_check=True)
            mm(ps[:, gyo[0]:gyo[1]], bC, xc, start=True, stop=False,
               skip_group_check=True)
            mm(ps[:, gyo[0]:gyo[1]], bsC, xmh, start=False, stop=False,
               skip_group_check=True)
            mm(ps[:, gyo[0]:gyo[1]], bsC, xph, start=False, stop=True,
               skip_group_check=True)

        for p in range(G // 2):          # pairs of groups
            ga = 2 * p
            xt = xpool.tile([128, 2 * GP], bf16)
            # zero the 4 pad cols of each 516-block: cols {0,1,514,515}
            pads = xt[:].rearrange("q (g j c) -> q g j c", g=2, c=BLK)
            nc.vector.memset(pads[:, :, :, 0:2], 0.0)
            nc.vector.memset(pads[:, :, :, 514:516], 0.0)
            # input DMA: one 3D strided start per group (4 row-tiles each).
            # dst data cols 2..513 per block; src rows g*512 + 126*j + q.
            for g2 in range(2):
                dst = pads[:, g2, :, 2:514]
                src = dram_ap(x_d[0:128, :],
                              [[W, 128], [126 * W, NT], [1, W]],
                              (ga + g2) * H * W)
                nc.sync.dma_start(out=dst, in_=src)

            ot = opool.tile([128, 2 * NT * W], mybir.dt.uint8)
            for g2 in range(2):
                g = ga + g2
                xg = xt[:, g2 * GP:(g2 + 1) * GP]
                nt = npool.tile([128, NT * W], bf16)
                gxs = gpool.tile([128, NT * W], bf16)
                gys = gpool.tile([128, NT * W], bf16)
                for j in range(NT):
                    var = 0 if j == 0 else 1
                    ps = ppool.tile([128, 1024], f32)
                    xb = xg[:, j * BLK:(j + 1) * BLK]
                    conv_tile(ps, (0, W), (W, 2 * W),
                              xb[:, 1:513], xb[:, 2:514], xb[:, 3:515],
                              band(0, var), band(1, var),
                              band(2, var), band(3, var))
                    # drain PSUM->SBUF (DVE may read only one PSUM input and
                    # runs 1x on f32): |gx| via ACT Abs (abs lives in every
                    # table set), gy via DVE copy (signed; sign cleared in
                    # the int16 AND pass below). ACT_DRAIN tiles put the gy
                    # drain on ACT too (load-balance knob).
                    nc.scalar.activation(gxs[:, j * W:(j + 1) * W],
                                         ps[:, 0:W], AF.Abs)
                    gyd = gys[:, j * W:(j + 1) * W]
                    if j in ACT_DRAIN:
                        nc.scalar.activation(gyd, ps[:, W:2 * W], AF.Abs)
                    else:
                        nc.vector.tensor_scalar(
                            out=gyd, in0=ps[:, W:2 * W], scalar1=0.0,
                            scalar2=None, op0=ALU.add)
                # clear sign bits of the DVE-drained gy halves (no-op on the
                # ACT-Abs ones) so max() below sees magnitudes
                nc.vector.tensor_scalar(
                    out=gys[:].bitcast(i16), in0=gys[:].bitcast(i16),
                    scalar1=0x7FFF, scalar2=None, op0=ALU.bitwise_and)
                # n = max(|gx|,|gy|)  (both SBUF bf16 -> 2x mode)
                nc.vector.tensor_tensor(out=nt[:, :], in0=gxs[:, :],
                                        in1=gys[:, :], op=ALU.max)
                # r ~= 1/(1.19 n):  bits(r) = MAGIC - bits(n)  (int16)
                rt = rpool.tile([128, NT * W], bf16)
                nc.vector.tensor_scalar(
                    out=rt[:].bitcast(i16), in0=nt[:].bitcast(i16),
                    scalar1=-1, scalar2=MAGIC,
                    op0=ALU.mult, op1=ALU.add)
                # v = x * r   (x0 slices, 4B-aligned -> 2x mode)
                vt = vpool.tile([128, NT * W], bf16)
                xg4 = xg.rearrange("q (j c) -> q j c", c=BLK)
                nc.vector.tensor_tensor(
                    out=vt[:].rearrange("q (j w) -> q j w", w=W),
                    in0=xg4[:, :, 2:514],
                    in1=rt[:].rearrange("q (j w) -> q j w", w=W),
                    op=ALU.mult)
                # g = atan(v) -> fp8 (pi/2 flip + affine folded on host)
                nc.scalar.activation(ot[:, g2 * 2048:(g2 + 1) * 2048].bitcast(fp8),
                                     vt[:, :], AF.Arctan)

            # output DMA: rows 1..126 of every tile are uniform (DRAM rows
            # 126*j + 1 + r, partitions 1..126) -> one 3D start per group.
            for g2 in range(2):
                src1 = dataclasses.replace(
                    ot[1:127, 0:W],
                    ap=[[2 * NT * W, 126], [W, NT], [1, W]],
                    offset=2 * NT * W + g2 * NT * W)
                dst1 = dram_ap(o_d[0:128, :],
                               [[W, 126], [126 * W, NT], [1, W]],
                               (ga + g2) * H * W + W)
                nc.sync.dma_start(out=dst1, in_=src1)
            # row 0 of each group (tile 0, partition 0) - plain slices on the
            # otherwise-idle GpSimd DGE ring
            for g2 in range(2):
                g = ga + g2
                nc.gpsimd.dma_start(
                    out=o_d[g * H:g * H + 1, :],
                    in_=ot[0:1, g2 * NT * W:g2 * NT * W + W])

        # ---- packed tail: rows 504..511 of all 12 groups on partitions
        # 8g+r, block-diagonal bottom bands, output rows 505..511. ----
        xtt = xpool.tile([128, BLK], bf16)
        nc.vector.memset(xtt[0:96, 0:2], 0.0)
        nc.vector.memset(xtt[0:96, 514:516], 0.0)
        for g in range(G):
            nc.gpsimd.dma_start(
                out=xtt[8 * g:8 * g + 8, 2:514],
                in_=x_d[g * H + TAIL_IN0:g * H + TAIL_IN0 + 8, :])

        pst = ppool.tile([128, 1024], f32)
        mm(pst[0:96, 0:W], tband(0), xtt[0:96, 3:515], start=True, stop=False,
           skip_group_check=True)
        mm(pst[0:96, 0:W], tband(1), xtt[0:96, 1:513], start=False, stop=True,
           skip_group_check=True)
        mm(pst[0:96, W:2 * W], tband(2), xtt[0:96, 2:514], start=True,
           stop=False, skip_group_check=True)
        mm(pst[0:96, W:2 * W], tband(3), xtt[0:96, 1:513], start=False,
           stop=False, skip_group_check=True)
        mm(pst[0:96, W:2 * W], tband(3), xtt[0:96, 3:515], start=False,
           stop=True, skip_group_check=True)
        gst = gpool.tile([128, W], bf16)
        gst2 = gpool.tile([128, W], bf16)
        nc.scalar.activation(gst[0:96, :], pst[0:96, 0:W], AF.Abs)
        nc.scalar.activation(gst2[0:96, :], pst[0:96, W:2 * W], AF.Abs)
        ntt = npool.tile([128, W], bf16)
        nc.vector.tensor_tensor(out=ntt[0:96, :], in0=gst[0:96, :],
                                in1=gst2[0:96, :], op=ALU.max)
        rtt = rpool.tile([128, W], bf16)
        nc.vector.tensor_scalar(
            out=rtt[0:96, :].bitcast(i16), in0=ntt[0:96, :].bitcast(i16),
            scalar1=-1, scalar2=MAGIC,
            op0=ALU.mult, op1=ALU.add)
        vtt = vpool.tile([128, W], bf16)
        nc.vector.tensor_tensor(out=vtt[0:96, :], in0=xtt[0:96, 2:514],
                                in1=rtt[0:96, :], op=ALU.mult)
        ott = opool.tile([128, W], mybir.dt.uint8)
        nc.scalar.activation(ott[0:96, :].bitcast(fp8), vtt[0:96, :], AF.Arctan)
        # store rows 505..511 (partitions 8g+1..8g+7) per group
        for g in range(G):
            nc.gpsimd.dma_start(
                out=o_d[g * H + TAIL_IN0 + 1:g * H + TAIL_IN0 + 8, :],
                in_=ott[8 * g + 1:8 * g + 8, 0:W])

    nc.compile()
    return nc


_NC_CACHE = {}


def _get_nc():
    if "nc" not in _NC_CACHE:
        _NC_CACHE["nc"] = build_nc()
    return _NC_CACHE["nc"]


def run(x: np.ndarray, trace: bool = False, **spmd_kwargs):
    """x: [32,3,512,512] f32 -> gabor [32,3,512,512] f32 (device part)."""
    import ml_dtypes
    from concourse.bass_utils import run_bass_kernel_spmd

    x = np.asarray(x)
    assert x.shape == (N_FULL, C, H, W), x.shape
    nc = _get_nc()
    bands = make_bands()
    shards = [
        np.ascontiguousarray(
            x[i * NPC:(i + 1) * NPC].reshape(G * H, W)).astype(
                ml_dtypes.bfloat16)
        for i in range(N_CORES)
    ]
    in_maps = [{"x": s, "bands": bands} for s in shards]
    res = run_bass_kernel_spmd(nc, in_maps, list(range(N_CORES)),
                               trace=trace, **spmd_kwargs)
    # host epilogue: fp8 g -> gabor = (pi/2 - g)/255 normalized per channel
    k1 = np.array([-1.0 / (255.0 * s) for s in STD], np.float32)
    k2 = np.array([(np.pi / 2.0 / 255.0 - m) / s for m, s in zip(MEAN, STD)],
                  np.float32)
    outs = []
    for i in range(N_CORES):
        gi = np.asarray(res.results[i]["gabor"])
        if gi.dtype == np.uint8:
            gi = gi.view(ml_dtypes.float8_e4m3)
        gi = gi.astype(np.float32).reshape(NPC, C, H, W)
        outs.append(gi * k1[None, :, None, None] + k2[None, :, None, None])
    gabor = np.concatenate(outs, axis=0)
    return gabor, res


def kernel(x: np.ndarray):
    xin = np.asarray(x)
    gabor, _ = run(xin)
    return (gabor, xin.astype(np.float32, copy=False))
